# revision 13
# baseline (speedup 1.0000x reference)
"""Causal single-head attention (B=4, S=4096, D=1024, H=128) on 8 NeuronCores.

Sharding: core c = (batch b = c//2, half h = c%2). Each core:
  - computes K^T [h, 4096] and V [4096, H] for its full batch row (replicated
    across the 2 cores of a batch),
  - handles 2048 query rows: 16 parity-interleaved 128-row subtiles
    (global subtile g = 8*r + 2*s + h for slot r in 0..3, s in 0..3),
  - slots have uniform causal k-tile limits [8, 16, 24, 32] so all 8 cores run
    the identical compiled program; causality is enforced with per-core mask
    DATA (qpos vs kiota is_ge compare) on the last 8 k-iters of each slot.

Mixed precision (validated vs fp32 reference, rel_err ~3.4e-3):
  - x cols 0:1024 + q-slot-0 columns arrive bf16; everything else fp8 e4m3
    (x pre-scaled x8, weights x64; 1/512 folded into the PSUM->SBUF copy)
  - K^T bricks 0..7, Q^T slot 0, V bricks 0..7 projected in bf16; all other
    projections fp8 DoubleRow (two 128-chunks contracted per matmul, 2x PE)
  - scores always bf16; P and V in fp8 for slots 1..3 (q >= 1024), where the
    PV matmul runs DoubleRow over k-tile pairs; slot 0 stays bf16
Denominator: dacc (bf16 DVE partial sums per k-lane) contracted against a
sqrt(H)-valued ones vector, one [128q,1] matmul per q-subtile -> fp32 PSUM,
then DVE reciprocal; O = transpose_bf16(O^T) * rec on DVE -> DMA.
DMA: per-128-row-chunk transfers (2-3KB descriptors), issue spread across
sync/scalar/vector/gpsimd queues, bf16 critical-path tensors first.
"""

import numpy as np
import ml_dtypes
from contextlib import ExitStack

import concourse.bass as bass
import concourse.tile as tile
from concourse import bacc, mybir
from concourse.bass_utils import run_bass_kernel_spmd

B, S, D, H = 4, 4096, 1024, 128
P = 128
BF16 = mybir.dt.bfloat16
F32 = mybir.dt.float32
F8 = mybir.dt.float8e4
NPBF16 = ml_dtypes.bfloat16
NPF8 = ml_dtypes.float8_e4m3

QLOC = 2048          # query rows per core
NSLOT = 4            # slots per core
SLOT_W = 512         # q columns per slot
LIMITS = [8, 16, 24, 32]   # k-tile limit per slot (same for every core)
NKT = S // P         # 32 k tiles
DCH = D // P         # 8 contraction chunks
SCALE = 1.0 / float(np.sqrt(H))     # pre-exp scale
SQRTH = float(np.sqrt(H))           # folded into denominator via ones vector
SX = 8.0             # host scale on x before fp8 cast
SW = 64.0            # host scale on W before fp8 cast
INVSXW = 1.0 / (SX * SW)
NB16 = 1024          # x columns (and q rows) kept in bf16
DR = mybir.MatmulPerfMode.DoubleRow


def qglob_for_core(h):
    """Global query row indices (length QLOC) handled by core-half h, in local order."""
    idx = []
    for r in range(NSLOT):
        for s in range(4):
            g = 8 * r + 2 * s + h
            idx.append(np.arange(g * P, (g + 1) * P))
    return np.concatenate(idx)


def build_nc():
    nc = bacc.Bacc(None, target_bir_lowering=False, debug=False, num_devices=8)

    xt16 = nc.dram_tensor("xt16", [D, NB16], BF16, kind="ExternalInput").ap()
    xt8 = nc.dram_tensor("xt8", [D, S - NB16], F8, kind="ExternalInput").ap()
    xqt16 = nc.dram_tensor("xqt16", [D, SLOT_W], BF16, kind="ExternalInput").ap()
    xqt8 = nc.dram_tensor("xqt8", [D, QLOC - SLOT_W], F8, kind="ExternalInput").ap()
    w16_ap, w8_ap = {}, {}
    for nm in ("wq", "wk", "wv"):
        w16_ap[nm] = nc.dram_tensor(nm + "16", [D, H], BF16, kind="ExternalInput").ap()
        w8_ap[nm] = nc.dram_tensor(nm + "8", [D, H], F8, kind="ExternalInput").ap()
    b_ap = {}
    for nm in ("bq", "bk", "bv"):
        b_ap[nm] = nc.dram_tensor(nm, [H, 1], F32, kind="ExternalInput").ap()
    qpos = nc.dram_tensor("qpos", [1, QLOC], mybir.dt.int16, kind="ExternalInput").ap()
    kio = nc.dram_tensor("kio", [P, NKT], mybir.dt.int16, kind="ExternalInput").ap()
    identb = nc.dram_tensor("identb", [P, P], BF16, kind="ExternalInput").ap()
    onesb = nc.dram_tensor("onesb", [P, 1], BF16, kind="ExternalInput").ap()
    out = nc.dram_tensor("out", [QLOC, H], F32, kind="ExternalOutput").ap()

    Ident = mybir.ActivationFunctionType.Identity
    Copy = mybir.ActivationFunctionType.Copy
    Exp = mybir.ActivationFunctionType.Exp

    with tile.TileContext(nc) as tc, ExitStack() as ctx:
        consts = ctx.enter_context(tc.tile_pool(name="consts", bufs=1))
        persist = ctx.enter_context(tc.tile_pool(name="persist", bufs=1))

        # ---- SBUF destinations
        w16_sb, w8_sb, b_sb = {}, {}, {}
        for nm in ("wq", "wk", "wv"):
            w16_sb[nm] = consts.tile([P, DCH, H], BF16, tag=f"w16_{nm}", name=f"w16_{nm}")
            w8_sb[nm] = consts.tile([P, DCH, H], F8, tag=f"w8_{nm}", name=f"w8_{nm}")
        for nm in ("bq", "bk", "bv"):
            b_sb[nm] = consts.tile([P, 1], F32, tag=f"b_{nm}", name=f"b_{nm}")
        qpos_b = consts.tile([P, QLOC], mybir.dt.int16, tag="qpos_b")
        kio_sb = consts.tile([P, NKT], mybir.dt.int16, tag="kio")
        identb_sb = consts.tile([P, P], BF16, tag="identb")
        ones_sb = consts.tile([P, 1], BF16, tag="ones")

        kT = persist.tile([P, S], BF16, tag="kT")             # K^T [h, s]
        vN16 = persist.tile([P, 8, H], BF16, tag="vN16")      # V natural, bricks 0..7
        vN8 = persist.tile([P, NKT, H], F8, tag="vN8")        # V natural fp8, all bricks
        qT = persist.tile([P, QLOC], BF16, tag="qT")          # Q^T [h, q_local]
        xt16_sb = persist.tile([P, DCH, NB16], BF16, tag="xt16_sb")
        xt8_sb = persist.tile([P, DCH, S - NB16], F8, tag="xt8_sb")
        xqt16_sb = persist.tile([P, DCH, SLOT_W], BF16, tag="xqt16_sb")
        xqt8_sb = persist.tile([P, DCH, QLOC - SLOT_W], F8, tag="xqt8_sb")

        # ---- DMA issue: chunk-PAIR transfers (fewer, bigger descriptors),
        # bf16 critical path fully ahead of fp8 bulk in every queue
        def load_pair(eng, dst, src, jp):
            eng.dma_start(
                out=dst[:, 2 * jp:2 * jp + 2, :],
                in_=src[2 * jp * P:(2 * jp + 2) * P, :].rearrange(
                    "(c p) s -> p c s", p=P),
            )

        def wload(eng, dst, src):
            eng.dma_start(out=dst[:], in_=src.rearrange("(c p) h -> p c h", p=P))

        # scalar: fp8 weights + first xt8 pairs (done before ACT's first op)
        wload(nc.scalar, w8_sb["wk"], w8_ap["wk"])
        wload(nc.scalar, w8_sb["wv"], w8_ap["wv"])
        load_pair(nc.scalar, xt8_sb, xt8, 0)
        load_pair(nc.scalar, xt8_sb, xt8, 1)
        wload(nc.scalar, w8_sb["wq"], w8_ap["wq"])

        # sync: first-projection critical path, then remaining fp8 bulk
        wload(nc.sync, w16_sb["wk"], w16_ap["wk"])
        wload(nc.sync, w16_sb["wv"], w16_ap["wv"])
        for nm in ("bk", "bv", "bq"):
            nc.sync.dma_start(out=b_sb[nm][:], in_=b_ap[nm])
        for jp in range(DCH // 2):
            load_pair(nc.sync, xt16_sb, xt16, jp)
        wload(nc.sync, w16_sb["wq"], w16_ap["wq"])
        for jp in range(DCH // 2):
            load_pair(nc.sync, xqt16_sb, xqt16, jp)
        nc.sync.dma_start(out=identb_sb[:], in_=identb)
        nc.sync.dma_start(out=ones_sb[:], in_=onesb)
        load_pair(nc.sync, xt8_sb, xt8, 2)
        load_pair(nc.sync, xt8_sb, xt8, 3)

        # gpsimd (SWDGE): xqt8, then mask tables (needed mid-attention only)
        for jp in range(DCH // 2):
            load_pair(nc.gpsimd, xqt8_sb, xqt8, jp)
        nc.gpsimd.dma_start(
            out=qpos_b[:],
            in_=bass.AP(tensor=qpos.tensor, offset=qpos.offset, ap=[[0, P], [1, QLOC]]),
        )
        nc.gpsimd.dma_start(out=kio_sb[:], in_=kio)

        # PSUM budget (8 banks): mm512 x3 + sT x3 + oT-pair x1(2 banks) = 8
        with tc.tile_pool(name="stg", bufs=3) as stg, \
             tc.tile_pool(name="mm", bufs=3, space="PSUM") as psA, \
             tc.tile_pool(name="psS", bufs=3, space="PSUM") as psS, \
             tc.tile_pool(name="psO", bufs=1, space="PSUM") as psO, \
             tc.tile_pool(name="pp", bufs=4) as pp, \
             tc.tile_pool(name="pp8", bufs=4) as pp8, \
             tc.tile_pool(name="acc", bufs=1) as accp, \
             tc.tile_pool(name="epi", bufs=3) as epi:

            def project16(src_sb, srs, wname):
                """bf16 weight-stationary projection of 512-col stripes of src."""
                pss = [psA.tile([P, SLOT_W], F32, tag="mm512", name=f"p16_{i}")
                       for i in range(len(srs))]
                for j in range(DCH):
                    for i, sr in enumerate(srs):
                        nc.tensor.matmul(
                            pss[i][:], lhsT=w16_sb[wname][:, j, :],
                            rhs=src_sb[:, j, sr * SLOT_W:(sr + 1) * SLOT_W],
                            start=(j == 0), stop=(j == DCH - 1),
                        )
                return pss

            def project8(src_sb, srs, wname):
                """fp8 DoubleRow projection (chunk pairs) of 512-col stripes."""
                pss = [psA.tile([P, SLOT_W], F32, tag="mm512", name=f"p8_{i}")
                       for i in range(len(srs))]
                for j in range(DCH // 2):
                    for i, sr in enumerate(srs):
                        nc.tensor.matmul(
                            pss[i][:], lhsT=w8_sb[wname][:, 2 * j:2 * j + 2, :],
                            rhs=src_sb[:, 2 * j:2 * j + 2, sr * SLOT_W:(sr + 1) * SLOT_W],
                            start=(j == 0), stop=(j == DCH // 2 - 1), perf_mode=DR,
                        )
                return pss

            def transpose_bricks(vTs, sr, dst):
                """PE-transpose a bf16 512-col V^T stripe into 4 natural bricks
                of dst (fp8 dst converts on the DVE copy)."""
                pst = psA.tile([P, SLOT_W], BF16, tag="mm512", name="vtr")
                for t_ in range(4):
                    nc.tensor.matmul(
                        pst[:, t_ * P:(t_ + 1) * P], lhsT=vTs[:, t_ * P:(t_ + 1) * P],
                        rhs=identb_sb[:], is_transpose=True, skip_group_check=True,
                    )
                nc.vector.tensor_copy(dst[:, sr * 4:(sr + 1) * 4, :], pst[:])

            def kv_stripes16():
                """K^T, V for stripes 0,1 (cols 0:1024) in bf16; vN8 copies too."""
                for ps, sr in zip(project16(xt16_sb, (0, 1), "wk"), (0, 1)):
                    nc.scalar.activation(
                        kT[:, sr * SLOT_W:(sr + 1) * SLOT_W], ps[:], Ident,
                        bias=b_sb["bk"][:], scale=1.0,
                    )
                for ps, sr in zip(project16(xt16_sb, (0, 1), "wv"), (0, 1)):
                    vTs = stg.tile([P, SLOT_W], BF16, tag="vT")
                    nc.scalar.activation(vTs[:], ps[:], Ident, bias=b_sb["bv"][:], scale=1.0)
                    transpose_bricks(vTs, sr, vN16)
                nc.vector.tensor_copy(vN8[:, 0:8, :], vN16[:])

            def kv_stripes8(*srs):
                """K^T, V for global stripes >= 2 via fp8 DoubleRow."""
                lsrs = [sr - 2 for sr in srs]   # xt8 local stripe index
                for ps, sr in zip(project8(xt8_sb, lsrs, "wk"), srs):
                    nc.scalar.activation(
                        kT[:, sr * SLOT_W:(sr + 1) * SLOT_W], ps[:], Ident,
                        bias=b_sb["bk"][:], scale=INVSXW,
                    )
                for ps, sr in zip(project8(xt8_sb, lsrs, "wv"), srs):
                    vTs = stg.tile([P, SLOT_W], BF16, tag="vT")
                    nc.scalar.activation(vTs[:], ps[:], Ident,
                                         bias=b_sb["bv"][:], scale=INVSXW)
                    transpose_bricks(vTs, sr, vN8)

            def q_slot0():
                (ps,) = project16(xqt16_sb, (0,), "wq")
                nc.scalar.activation(qT[:, 0:SLOT_W], ps[:], Ident,
                                     bias=b_sb["bq"][:], scale=1.0)

            def q_slots8(*qrs):
                lqs = [qr - 1 for qr in qrs]   # xqt8 local stripe index
                for ps, qr in zip(project8(xqt8_sb, lqs, "wq"), qrs):
                    nc.scalar.activation(
                        qT[:, qr * SLOT_W:(qr + 1) * SLOT_W], ps[:], Ident,
                        bias=b_sb["bq"][:], scale=INVSXW,
                    )

            def attention_pass(slots):
                """kt-pair-outer attention over a pair of slots (shared K/V)."""
                Ls = {r: LIMITS[r] for r in slots}
                Tmax = max(Ls.values()) // 2
                oT = psO.tile([P, len(slots), SLOT_W], F32, tag="oT")
                dacc = {r: accp.tile([P, SLOT_W], BF16, tag=f"dacc{r}", name=f"dacc{r}")
                        for r in slots}

                def c0_of(r, kt):
                    # first column (h-safe) any core's subtile can still attend
                    # at this k-brick; earlier columns are masked for both
                    # halves and skipped (fp8 slots use the even-kt value for
                    # the whole pair; the mask zeroes the stale subtile)
                    return P * max(0, (kt - 8 * r) // 2)

                def dacc_upd(r, src_ap, c0, first, pool=False):
                    if first:
                        nc.vector.tensor_copy(dacc[r][:], src_ap)
                    else:
                        eng = nc.gpsimd if pool else nc.vector
                        eng.tensor_add(dacc[r][:, c0:], dacc[r][:, c0:], src_ap)

                def score_one(r, kt, dst_ap, c0):
                    """S^T matmul + exp (+ causal mask) into dst_ap (pre-sliced at c0)."""
                    qsl = slice(r * SLOT_W + c0, (r + 1) * SLOT_W)
                    sT = psS.tile([P, SLOT_W], F32, tag="sT")
                    nc.tensor.matmul(
                        sT[:, c0:], lhsT=kT[:, kt * P:(kt + 1) * P], rhs=qT[:, qsl],
                        start=True, stop=True,
                    )
                    nc.scalar.activation(dst_ap, sT[:, c0:], Exp, scale=SCALE)
                    if kt >= Ls[r] - 8:
                        # p = (qpos >= kpos) * p  (fused causal mask); only the
                        # two subtiles at c0 can straddle/trail the diagonal
                        w = min(2 * P, SLOT_W - c0)
                        nc.vector.scalar_tensor_tensor(
                            dst_ap[:, :w], qpos_b[:, qsl.start:qsl.start + w],
                            kio_sb[:, kt:kt + 1], dst_ap[:, :w],
                            op0=mybir.AluOpType.is_ge, op1=mybir.AluOpType.mult,
                        )

                def score_pair(t):
                    outs = {}
                    for r in slots:
                        if 2 * t >= Ls[r]:
                            continue
                        c0 = c0_of(r, 2 * t)
                        if r == 0:
                            pTs = []
                            for i in range(2):
                                pT = pp.tile([P, SLOT_W], BF16, tag="pT")
                                score_one(r, 2 * t + i, pT[:, c0:], c0)
                                dacc_upd(r, pT[:, c0:], c0, first=(t == 0 and i == 0))
                                pTs.append(pT)
                            outs[r] = (pTs, c0)
                        else:
                            pT8 = pp8.tile([P, 2, SLOT_W], F8, tag="pT8")
                            for i in range(2):
                                score_one(r, 2 * t + i, pT8[:, i, c0:], c0)
                                dacc_upd(r, pT8[:, i, c0:], c0,
                                         first=(t == 0 and i == 0),
                                         pool=(i == 1 and r >= 2))
                            outs[r] = (pT8, c0)
                    return outs

                def accum_pair(t, outs):
                    for i_s, r in enumerate(slots):
                        if r not in outs:
                            continue
                        buf, c0 = outs[r]
                        if r == 0:
                            for i in range(2):
                                nc.tensor.matmul(
                                    oT[:, i_s, c0:], lhsT=vN16[:, 2 * t + i, :],
                                    rhs=buf[i][:, c0:],
                                    start=(t == 0 and i == 0),
                                    stop=(2 * t + i == Ls[r] - 1),
                                )
                        else:
                            nc.tensor.matmul(
                                oT[:, i_s, c0:], lhsT=vN8[:, 2 * t:2 * t + 2, :],
                                rhs=buf[:, :, c0:],
                                start=(t == 0), stop=(2 * t + 1 == Ls[r] - 1),
                                perf_mode=DR,
                            )

                def epilogue(i_s, r):
                    """O = transpose(O^T) / (d * sqrt(H)) for one slot."""
                    d_ps = psA.tile([P, 4], F32, tag="mm512", name="dmm")
                    for s_ in range(4):
                        nc.tensor.matmul(
                            d_ps[:, s_:s_ + 1],
                            lhsT=dacc[r][:, s_ * P:(s_ + 1) * P], rhs=ones_sb[:],
                            start=(s_ == 0), stop=(s_ == 3), skip_group_check=True,
                        )
                    oTs = epi.tile([P, SLOT_W], BF16, tag="oTs")
                    nc.scalar.activation(oTs[:], oT[:, i_s, :], Copy)
                    rec = epi.tile([P, 4], F32, tag="rec")
                    nc.vector.reciprocal(rec[:], d_ps[:])
                    obr = psA.tile([P, SLOT_W], BF16, tag="mm512", name="obr")
                    for s_ in range(4):
                        nc.tensor.matmul(
                            obr[:, s_ * P:(s_ + 1) * P], lhsT=oTs[:, s_ * P:(s_ + 1) * P],
                            rhs=identb_sb[:], is_transpose=True, skip_group_check=True,
                        )
                    ofin = epi.tile([P, SLOT_W], F32, tag="ofin")
                    for s_ in range(4):
                        nc.vector.tensor_scalar_mul(
                            ofin[:, s_ * P:(s_ + 1) * P], obr[:, s_ * P:(s_ + 1) * P],
                            rec[:, s_:s_ + 1],
                        )
                    nc.sync.dma_start(
                        out=out[r * SLOT_W:(r + 1) * SLOT_W, :].rearrange(
                            "(s p) h -> p s h", p=P
                        ),
                        in_=ofin[:].rearrange("p (s h) -> p s h", s=4),
                    )

                prev = score_pair(0)
                for t in range(1, Tmax):
                    cur = score_pair(t)
                    accum_pair(t - 1, prev)
                    prev = cur
                    # emit the shorter slot's epilogue as soon as it stops
                    for i_s, r in enumerate(slots):
                        if Ls[r] == 2 * t:
                            epilogue(i_s, r)
                accum_pair(Tmax - 1, prev)
                for i_s, r in enumerate(slots):
                    if Ls[r] == 2 * Tmax:
                        epilogue(i_s, r)

            # emission: all projections first (their PSUM slot allocations must
            # not queue behind pass epilogues), then the attention passes
            kv_stripes16()
            q_slot0()
            kv_stripes8(2, 3)
            q_slots8(1)
            kv_stripes8(4, 5)
            kv_stripes8(6, 7)
            q_slots8(2, 3)
            attention_pass((0, 1))
            attention_pass((2, 3))

    nc.compile()
    return nc


_NC_CACHE = None


def _get_nc():
    global _NC_CACHE
    if _NC_CACHE is None:
        _NC_CACHE = build_nc()
    return _NC_CACHE


def make_in_maps(inputs):
    x = np.asarray(inputs["x"], np.float32)
    Ws = {nm: np.asarray(inputs[Wnm], np.float32)
          for nm, Wnm in (("wq", "Wq"), ("wk", "Wk"), ("wv", "Wv"))}
    bs = {nm: np.asarray(inputs[bnm], np.float32).reshape(H, 1)
          for nm, bnm in (("bq", "bq"), ("bk", "bk"), ("bv", "bv"))}

    kio = (np.arange(NKT)[None, :] * P + np.arange(P)[:, None]).astype(np.int16)
    common = dict(
        kio=kio,
        identb=np.eye(P, dtype=NPBF16),
        onesb=np.full((P, 1), SQRTH, dtype=NPBF16),
        **bs,
    )
    for nm, W in Ws.items():
        common[nm + "16"] = W.astype(NPBF16)
        common[nm + "8"] = (W * SW).astype(NPF8)

    in_maps = []
    xT = np.ascontiguousarray(x.transpose(0, 2, 1))        # [B, D, S] fp32
    xT16 = xT.astype(NPBF16)
    xT8 = (xT * SX).astype(NPF8)
    for c in range(8):
        b, hh = c // 2, c % 2
        qg = qglob_for_core(hh)
        m = dict(common)
        m["xt16"] = xT16[b][:, :NB16]
        m["xt8"] = np.ascontiguousarray(xT8[b][:, NB16:])
        m["xqt16"] = np.ascontiguousarray(xT16[b][:, qg[:SLOT_W]])
        m["xqt8"] = np.ascontiguousarray(xT8[b][:, qg[SLOT_W:]])
        m["qpos"] = qg.astype(np.int16).reshape(1, QLOC)
        in_maps.append(m)
    return in_maps


def assemble_out(results):
    out = np.zeros((1, B, S, H), np.float32)
    for c in range(8):
        b, hh = c // 2, c % 2
        qg = qglob_for_core(hh)
        out[0, b, qg, :] = results[c]["out"]
    return out


def kernel(**inputs) -> np.ndarray:
    nc = _get_nc()
    in_maps = make_in_maps(inputs)
    res = run_bass_kernel_spmd(nc, in_maps, list(range(8)))
    return assemble_out(res.results)
